# revision 1
# baseline (speedup 1.0000x reference)
"""Fused co-memory cross-attention kernel for Trainium2, SPMD over 8 NeuronCores.

Module: LayerNorm(q/k/v) -> per-head projections -> masked softmax attention
        -> output projection.  B=2, Sq=1024, Sk=5*1024, C=256, 8 heads x 32.

Sharding: data-parallel over batch (2) x query-block parallel (4) = 8 cores.
Each core handles 256 query rows of one batch against that batch's full
(mask-compacted) key/value set.  The frame mask is exploited on the host:
masked frames are dropped before they ever reach the device (sparse
attention), with -1e9 bias padding frames only to equalize the SPMD shape
across batches.

Device kernel (per core), all fused on-chip (scores never touch HBM), fp16
data path with fp32 statistics/accumulation (rel err ~8e-4):
  - LN via bn_stats/bn_aggr; gamma, beta and the 1/sqrt(d) score scale are
    folded into the projection weights host-side; rstd is computed on
    VectorE (quake seed + 2 Newton steps) so ScalarE keeps only the Exp
    table resident (table-set switches cost ~1.3us each)
  - activations are PE-transposed to C-on-partition layout for projections
  - scores^T per (head, sk-tile) as K=32 matmuls, head-PAIR packed on array
    row strips {0,32} with outputs in different PSUM banks: two concurrent
    partial-row matmuls writing the same PSUM bank at the same partitions
    hard-fault the exec unit (found empirically), same-strip matmuls
    serialize in-array which makes bank reuse safe
  - softmax without max-subtraction (LN-bounded scores); exp on ScalarE
    batched over 4 heads x 1024 elems, frame mask bias applied through the
    activation bias port (padding frames -> exp(-1e9) == 0)
  - PV via col-strip (M=32) matmuls, 4 heads concurrent into one PSUM bank
    at disjoint partition ranges; denominators via ones-vector matmuls
    (N=512, two per tile) accumulating alongside
  - normalize (reciprocal + DRAM-bounce partition broadcast) + output
    projection on-chip; only the final [256,256] fp32 slice leaves the core
"""

import math
import os

import numpy as np

HEADS = 8
KD = 32
C = 256
EPS = 1e-3
B = 2
SQ = 1024          # queries per batch (Tq*H*W)
FTOK = 1024        # tokens per memory frame (KH*KW)
TK = 5
NCORES = 8
QPC = SQ // (NCORES // B)   # 256 query rows per core
NEG = -1.0e9
P = 128

_cache: dict = {}

# Set by kernel() when BASS_KERNEL_TRACE=1: HW exec time of the slowest
# profiled core, in ns (or None if profiling unavailable).
last_exec_time_ns = None
last_results = None


def _build_program(F: int, use_tk: bool, fp16: bool):
    stage = int(os.environ.get("KERNEL_STAGE", "5"))
    attn_mode = os.environ.get("KERNEL_ATTN", "full")
    from contextlib import ExitStack

    import concourse.bass as bass
    import concourse.tile as tile
    from concourse import bacc, mybir
    from concourse.masks import make_identity

    dt = mybir.dt
    f32 = dt.float32
    mdt = dt.float16 if fp16 else dt.float32
    AF = mybir.ActivationFunctionType
    OP = mybir.AluOpType
    SK = F * FTOK
    NT = SK // P             # sk token tiles of 128
    NCH = NT // 2            # 256-token chunks

    nc = bacc.Bacc("TRN2", target_bir_lowering=False, debug=False,
                   num_devices=NCORES)

    xq_d = nc.dram_tensor("xq", [QPC, C], mdt, kind="ExternalInput").ap()
    k_d = nc.dram_tensor("kin", [SK, C], mdt, kind="ExternalInput").ap()
    v_d = nc.dram_tensor("vin", [SK, C], mdt, kind="ExternalInput").ap()
    wq_d = nc.dram_tensor("wq", [C, C], mdt, kind="ExternalInput").ap()
    wk_d = nc.dram_tensor("wk", [C, C], mdt, kind="ExternalInput").ap()
    wv_d = nc.dram_tensor("wv", [C, C], mdt, kind="ExternalInput").ap()
    wo_d = nc.dram_tensor("wo", [C, C], mdt, kind="ExternalInput").ap()
    tq_d = nc.dram_tensor("tq", [C, 1], f32, kind="ExternalInput").ap()
    tk_d = nc.dram_tensor("tkc", [C, 1], f32, kind="ExternalInput").ap()
    tv_d = nc.dram_tensor("tv", [C, 1], f32, kind="ExternalInput").ap()
    fb_d = nc.dram_tensor("fbias", [1, F], f32, kind="ExternalInput").ap()
    out_d = nc.dram_tensor("out", [QPC, C], f32, kind="ExternalOutput").ap()
    dsc_d = nc.dram_tensor("den_scratch", [2, 4, 2 * P], f32).ap()

    with tile.TileContext(nc) as tc, ExitStack() as ctx:
        singles = ctx.enter_context(tc.tile_pool(name="singles", bufs=1))
        stats_p = ctx.enter_context(tc.tile_pool(name="stats", bufs=4))
        io_p = ctx.enter_context(tc.tile_pool(name="io", bufs=4))
        xh_p = ctx.enter_context(tc.tile_pool(name="xh", bufs=3))
        chunk_p = ctx.enter_context(tc.tile_pool(name="chunk", bufs=3))
        exp_p = ctx.enter_context(tc.tile_pool(name="exp", bufs=4))
        out_p = ctx.enter_context(tc.tile_pool(name="outp", bufs=2))
        ps_small = ctx.enter_context(
            tc.tile_pool(name="ps_small", bufs=2, space="PSUM"))
        ps_sc = ctx.enter_context(
            tc.tile_pool(name="ps_sc", bufs=2, space="PSUM"))
        ps_acc = ctx.enter_context(
            tc.tile_pool(name="ps_acc", bufs=1, space="PSUM"))

        # ---- constants / weights ----
        ident = singles.tile([P, P], mdt)
        make_identity(nc, ident[:])
        ones_t = singles.tile([P, 1], mdt)
        nc.vector.memset(ones_t[:], 1.0)
        eps_t = singles.tile([P, 1], f32)
        nc.vector.memset(eps_t[:], EPS)
        fb_t = singles.tile([P, F], f32)
        nc.sync.dma_start(
            out=fb_t[:],
            in_=bass.AP(tensor=fb_d.tensor, offset=fb_d.offset,
                        ap=[[0, P], [1, F]]))

        w_tiles = {}
        for name, d in (("wq", wq_d), ("wk", wk_d), ("wv", wv_d), ("wo", wo_d)):
            for kt in range(2):
                t = singles.tile([P, C], mdt, tag=f"{name}{kt}")
                nc.sync.dma_start(out=t[:], in_=d[kt * P:(kt + 1) * P, :])
                w_tiles[(name, kt)] = t

        def load_col(dram_ap, tag):
            t = singles.tile([P, 2], f32, tag=tag)
            nc.sync.dma_start(
                out=t[:], in_=dram_ap.rearrange("(a p) o -> p (a o)", p=P))
            return t

        tq_t = load_col(tq_d, "tq")
        tv_t = load_col(tv_d, "tv")
        tk_t = load_col(tk_d, "tk") if use_tk else None

        def dbg_out(tiles):
            for qt, tl in enumerate(tiles):
                ot = out_p.tile([P, C], f32, tag="ot", name="dbg")
                nc.vector.tensor_copy(ot[:], tl)
                nc.sync.dma_start(out=out_d[qt * P:(qt + 1) * P, :], in_=ot[:])

        if stage <= 1:
            dbg_out([w_tiles[("wq", 0)][:], w_tiles[("wq", 1)][:]])

        # ---- LayerNorm, batched over up to 4 tiles.
        # rstd = rsqrt(var+eps) computed entirely on VectorE (quake seed +
        # two Newton steps) so ScalarE keeps only the softmax Exp table
        # resident for the whole kernel.
        i32 = dt.int32

        def ln_stats(x_tiles):
            n = len(x_tiles)
            mvb = stats_p.tile([P, 4, 2], f32, tag="mv", name="mvb")
            for i, x_t in enumerate(x_tiles):
                st = stats_p.tile([P, 6], f32, tag="bn", name="st")
                nc.vector.bn_stats(st[:], x_t[:])
                nc.vector.bn_aggr(mvb[:, i, :], st[:])
            ve = stats_p.tile([P, 4], f32, tag="ve", name="ve")
            nc.vector.tensor_scalar(ve[:, 0:n], mvb[:, 0:n, 1], scalar1=EPS,
                                    scalar2=None, op0=OP.add)
            y = stats_p.tile([P, 4], f32, tag="y0", name="y0")
            nc.vector.tensor_scalar(y[:, 0:n].bitcast(i32),
                                    ve[:, 0:n].bitcast(i32),
                                    scalar1=1, scalar2=None,
                                    op0=OP.logical_shift_right)
            nc.vector.tensor_scalar(y[:, 0:n].bitcast(i32),
                                    y[:, 0:n].bitcast(i32),
                                    scalar1=-1, scalar2=0x5F3759DF,
                                    op0=OP.mult, op1=OP.add)
            rstdb = y
            for _ in range(2):
                c = stats_p.tile([P, 4], f32, tag="nc", name="nwt")
                nc.vector.tensor_mul(c[:, 0:n], y[:, 0:n], y[:, 0:n])
                nc.vector.tensor_mul(c[:, 0:n], c[:, 0:n], ve[:, 0:n])
                nc.vector.tensor_scalar(c[:, 0:n], c[:, 0:n], scalar1=-0.5,
                                        scalar2=1.5, op0=OP.mult, op1=OP.add)
                yn = stats_p.tile([P, 4], f32, tag="yn", name="yn")
                nc.vector.tensor_mul(yn[:, 0:n], y[:, 0:n], c[:, 0:n])
                y = yn
                rstdb = yn
            return mvb, rstdb

        def ln_apply(x_t, mvb, rstdb, i):
            nmr = stats_p.tile([P, 1], f32, tag="nmr", name="nmr")
            nc.vector.tensor_scalar(nmr[:], mvb[:, i, 0:1],
                                    scalar1=rstdb[:, i:i + 1],
                                    scalar2=-1.0, op0=OP.mult, op1=OP.mult)
            xh = xh_p.tile([P, C], mdt, tag="xh")
            nc.vector.tensor_scalar(xh[:], x_t[:], scalar1=rstdb[:, i:i + 1],
                                    scalar2=nmr[:], op0=OP.mult, op1=OP.add)
            return xh

        tr_count = [0]

        def transpose_128(dst_ap, src_ap):
            pt = ps_small.tile([P, P], mdt, tag="ps", name="pt")
            nc.tensor.transpose(pt[:], src_ap, ident[:])
            # alternate drain engine: VectorE is the busier engine overall
            if tr_count[0] % 2 == 0:
                nc.vector.tensor_copy(dst_ap, pt[:])
            else:
                nc.scalar.copy(dst_ap, pt[:])
            tr_count[0] += 1

        # ---- Q path ----
        xqT = [singles.tile([P, 2 * P], mdt, tag=f"xqT{i}", name=f"xqT{i}")
               for i in range(2)]
        if stage >= 2:
            xts = []
            for qt in range(2):
                xt = io_p.tile([P, C], mdt, tag="xq", name="xt")
                nc.sync.dma_start(out=xt[:], in_=xq_d[qt * P:(qt + 1) * P, :])
                xts.append(xt)
            mvb, rstdb = ln_stats(xts)
            for qt in range(2):
                xh = ln_apply(xts[qt], mvb, rstdb, qt)
                for ct in range(2):
                    transpose_128(xqT[ct][:, qt * P:(qt + 1) * P],
                                  xh[:, ct * P:(ct + 1) * P])
        # qp[i] holds heads {2i, 2i+1} on partitions 0-63: score matmuls for
        # a head pair use array row strips {0, 32}, so concurrent strips never
        # write the same PSUM bank (same-strip matmuls serialize in-array).
        qp = [singles.tile([64, 2 * P], mdt, tag=f"qp{i}", name=f"qp{i}")
              for i in range(4)]
        for mt in range(2 if stage >= 2 else 0):
            ps = ps_small.tile([P, 2 * P], f32, tag="ps")
            for kt in range(2):
                nc.tensor.matmul(
                    ps[:], w_tiles[("wq", kt)][:, mt * P:(mt + 1) * P],
                    xqT[kt][:], start=(kt == 0), stop=(kt == 1))
            for half in range(2):
                nc.vector.tensor_scalar(
                    qp[2 * mt + half][:], ps[64 * half:64 * half + 64, :],
                    scalar1=tq_t[64 * half:64 * half + 64, mt:mt + 1],
                    scalar2=None, op0=OP.add)

        if stage == 2:
            dbg_out([qp[0][:].partition_broadcast(2), qp[1][:].partition_broadcast(2)])

        # ---- K/V path: LN -> transpose -> project ----
        kp = [singles.tile([64, SK], mdt, tag=f"kp{i}", name=f"kp{i}")
              for i in range(4)]
        vh = singles.tile([P, NT, C], mdt, tag="vh")
        for ch in range(NCH if stage >= 3 else 0):
            kT = chunk_p.tile([P, 2, 2 * P], mdt, tag="kT")
            vT = chunk_p.tile([P, 2, 2 * P], mdt, tag="vT")
            xts = []
            for j in range(2):
                t = 2 * ch + j
                for src_d, eng in ((k_d, nc.gpsimd), (v_d, nc.sync)):
                    xt = io_p.tile([P, C], mdt, tag="kv_in", name="xt")
                    eng.dma_start(out=xt[:],
                                  in_=src_d[t * P:(t + 1) * P, :])
                    xts.append(xt)
            mvb, rstdb = ln_stats(xts)
            for idx in range(4):
                j, dstT = idx // 2, (kT, vT)[idx % 2]
                xh = ln_apply(xts[idx], mvb, rstdb, idx)
                for ct in range(2):
                    transpose_128(dstT[:, ct, j * P:(j + 1) * P],
                                  xh[:, ct * P:(ct + 1) * P])
            # k projection -> khT (C_out on partitions, tokens on free)
            for mt in range(2):
                ps = ps_small.tile([P, 2 * P], f32, tag="ps")
                for kt in range(2):
                    nc.tensor.matmul(
                        ps[:],
                        w_tiles[("wk", kt)][:, mt * P:(mt + 1) * P],
                        kT[:, kt, :],
                        start=(kt == 0), stop=(kt == 1))
                for half in range(2):
                    dst = kp[2 * mt + half][:, ch * 2 * P:(ch + 1) * 2 * P]
                    srcp = ps[64 * half:64 * half + 64, :]
                    if use_tk:
                        nc.vector.tensor_scalar(
                            dst, srcp,
                            scalar1=tk_t[64 * half:64 * half + 64, mt:mt + 1],
                            scalar2=None, op0=OP.add)
                    else:
                        nc.scalar.copy(dst, srcp)
            # v projection -> vh (tokens on partitions, C_out on free)
            for j in range(2):
                ps = ps_small.tile([P, C], f32, tag="ps")
                for kt in range(2):
                    nc.tensor.matmul(
                        ps[:], vT[:, kt, j * P:(j + 1) * P],
                        w_tiles[("wv", kt)][:],
                        start=(kt == 0), stop=(kt == 1))
                nc.scalar.copy(vh[:, 2 * ch + j, :], ps[:])

        if stage == 3:
            dbg_out([kp[0][:, 0:C].partition_broadcast(2), kp[1][:, 0:C].partition_broadcast(2)])

        # ---- attention: per quad of heads, stream sk tiles ----
        ctxn = [singles.tile([P, 2 * P], mdt, tag=f"ctxn{q}", name=f"ctxn{q}")
                for q in range(2)]
        for quad in range(2 if stage >= 4 else 0):
            ctx_ps = ps_acc.tile([P, 2 * P], f32, tag="ctx", name="ctx_ps")
            den_ps = ps_acc.tile([P, 4 * P], f32, tag="den", name="den_ps")
            for f in range(F):
                for st in range(8):
                    t = 8 * f + st
                    sc = ps_sc.tile([P, 4, 2 * P], f32, tag="sc")
                    for j in range(4):
                        pr, e = j // 2, j % 2
                        nc.tensor.matmul(
                            sc[:, 2 * e + pr, :],
                            kp[2 * quad + pr][32 * e:32 * e + 32,
                                              t * P:(t + 1) * P],
                            qp[2 * quad + pr][32 * e:32 * e + 32, :],
                            start=True, stop=True, tile_position=(32 * e, 0),
                            skip_group_check=True)
                    ex = exp_p.tile([P, 4, 2 * P], mdt, tag="exp")
                    nc.scalar.activation(ex[:], sc[:], AF.Exp,
                                         bias=fb_t[:, f:f + 1])
                    if attn_mode == "sc":
                        if t == 0:
                            nc.vector.tensor_copy(ctxn[quad][:], ex[:, 0, :])
                        continue
                    for j in range(4 if attn_mode != "j3" else 3):
                        h = 4 * quad + j
                        slot = 2 * (j % 2) + j // 2
                        nc.tensor.matmul(
                            ctx_ps[32 * j:32 * j + 32, :],
                            vh[:, t, 32 * h:32 * h + 32],
                            ex[:, slot, :],
                            start=(t == 0), stop=(t == NT - 1),
                            tile_position=(0, 32 * j), skip_group_check=True)
                    if attn_mode not in ("pv", "j3", "sc"):
                        for hb in range(2):
                            nc.tensor.matmul(
                                den_ps[32 * hb:32 * hb + 1, :],
                                ones_t[:],
                                ex[:, 2 * hb:2 * hb + 2, :],
                                start=(t == 0), stop=(t == NT - 1),
                                tile_position=(0, 32 * hb),
                                skip_group_check=True)
            if attn_mode != "full":
                continue
            # normalize: ctx / denom (+ beta_m @ Wv correction)
            den_sb = out_p.tile([P, 2 * P], f32, tag="den_sb")
            for j in range(4):
                hb, s = j % 2, j // 2
                nc.vector.tensor_copy(
                    den_sb[32 * j:32 * j + 1, :],
                    den_ps[32 * hb:32 * hb + 1, 2 * P * s:2 * P * (s + 1)])
            nc.sync.dma_start(
                out=dsc_d[quad],
                in_=bass.AP(tensor=den_sb.tensor, offset=den_sb.offset,
                            ap=[[32 * den_sb.shape[-1], 4], [1, 2 * P]]))
            rden = out_p.tile([P, 2 * P], f32, tag="rden")
            for j in range(4):
                nc.sync.dma_start(
                    out=rden[32 * j:32 * j + 32, :],
                    in_=dsc_d[quad, j:j + 1, :].partition_broadcast(32))
            nc.vector.reciprocal(rden[:], rden[:])
            nc.vector.tensor_mul(ctxn[quad][:], ctx_ps[:], rden[:])
            nc.vector.tensor_scalar(ctxn[quad][:], ctxn[quad][:],
                                    scalar1=tv_t[:, quad:quad + 1],
                                    scalar2=None, op0=OP.add)

        if stage == 4:
            dbg_out([ctxn[0][:], ctxn[1][:]])

        # ---- output projection ----
        for qt in range(2 if stage >= 5 else 0):
            ps = ps_small.tile([P, C], f32, tag="ps")
            for kt in range(2):
                nc.tensor.matmul(
                    ps[:], ctxn[kt][:, qt * P:(qt + 1) * P],
                    w_tiles[("wo", kt)][:],
                    start=(kt == 0), stop=(kt == 1))
            ot = out_p.tile([P, C], f32, tag="ot")
            nc.vector.tensor_copy(ot[:], ps[:])
            nc.sync.dma_start(out=out_d[qt * P:(qt + 1) * P, :], in_=ot[:])

    nc.compile()
    return nc


def _get_program(F: int, use_tk: bool, fp16: bool = True):
    key = (F, use_tk, fp16, os.environ.get("KERNEL_STAGE", "5"),
           os.environ.get("KERNEL_ATTN", "full"))
    if key not in _cache:
        _cache[key] = _build_program(F, use_tk, fp16)
    return _cache[key]


def _prep_host(encoder_output, memory_key, memory_value, Wq, Wk, Wv, Wo,
               gamma_q, beta_q, gamma_m, beta_m, memory_mask, fp16=True):
    f32 = np.float32
    mdt = np.float16 if fp16 else np.float32
    enc = np.ascontiguousarray(
        np.asarray(encoder_output, dtype=f32).reshape(B, SQ, C))
    mk = np.asarray(memory_key, dtype=f32).reshape(B, TK, FTOK, C)
    mv = np.asarray(memory_value, dtype=f32).reshape(B, TK, FTOK, C)
    mask = np.asarray(memory_mask).astype(np.int64)

    gq = np.asarray(gamma_q, dtype=f32)
    bq = np.asarray(beta_q, dtype=f32)
    gm = np.asarray(gamma_m, dtype=f32)
    bm = np.asarray(beta_m, dtype=f32)
    Wq = np.asarray(Wq, dtype=f32)
    Wk = np.asarray(Wk, dtype=f32)
    Wv = np.asarray(Wv, dtype=f32)
    Wo = np.ascontiguousarray(np.asarray(Wo, dtype=f32))

    s = 1.0 / math.sqrt(KD)
    wq2 = np.ascontiguousarray(gq[:, None] * Wq * s)
    tq = np.ascontiguousarray((bq @ Wq * s).reshape(C, 1))
    wk2 = np.ascontiguousarray(gm[:, None] * Wk)
    tkc = np.ascontiguousarray((bm @ Wk).reshape(C, 1))
    wv2 = np.ascontiguousarray(gm[:, None] * Wv)
    tv = np.ascontiguousarray((bm @ Wv).reshape(C, 1))
    use_tk = bool(np.any(tkc != 0.0))

    # frame selection per batch
    sel = []        # list of (frame_indices, fbias, uniform_mode)
    counts = []
    for b in range(B):
        act = np.nonzero(mask[b])[0]
        if len(act) == 0:
            sel.append((list(range(TK)), None, True))
            counts.append(TK)
        else:
            sel.append((list(act), None, False))
            counts.append(len(act))
    F = max(counts)

    per_batch = []
    for b in range(B):
        frames, _, uniform = sel[b]
        fb = np.zeros((1, F), dtype=f32)
        fr = list(frames)
        while len(fr) < F:
            fr.append(frames[-1])
            fb[0, len(fr) - 1] = NEG
        kb = np.ascontiguousarray(mk[b][fr].reshape(F * FTOK, C))
        vb = np.ascontiguousarray(mv[b][fr].reshape(F * FTOK, C))
        if uniform:
            wq_b = np.zeros_like(wq2)
            tq_b = np.zeros_like(tq)
        else:
            wq_b = wq2
            tq_b = tq
        per_batch.append(dict(kin=kb.astype(mdt), vin=vb.astype(mdt),
                              wq=wq_b.astype(mdt), tq=tq_b, fbias=fb))

    in_maps = []
    for c in range(NCORES):
        b = c // (NCORES // B)
        qs = c % (NCORES // B)
        m = dict(per_batch[b])
        m["xq"] = np.ascontiguousarray(enc[b, qs * QPC:(qs + 1) * QPC]).astype(mdt)
        m["wk"] = wk2.astype(mdt)
        m["wv"] = wv2.astype(mdt)
        m["wo"] = Wo.astype(mdt)
        m["tkc"] = tkc
        m["tv"] = tv
        in_maps.append(m)
    return F, use_tk, in_maps


def kernel(encoder_output, memory_key, memory_value, Wq, Wk, Wv, Wo,
           gamma_q, beta_q, gamma_m, beta_m, memory_mask):
    global last_exec_time_ns, last_results
    from concourse.bass_utils import run_bass_kernel_spmd

    fp16 = os.environ.get("KERNEL_FP32", "0") != "1"
    F, use_tk, in_maps = _prep_host(
        encoder_output, memory_key, memory_value, Wq, Wk, Wv, Wo,
        gamma_q, beta_q, gamma_m, beta_m, memory_mask, fp16=fp16)
    nc = _get_program(F, use_tk, fp16)

    trace = os.environ.get("BASS_KERNEL_TRACE", "0") == "1"
    res = run_bass_kernel_spmd(nc, in_maps, core_ids=list(range(NCORES)),
                               trace=trace)
    last_exec_time_ns = res.exec_time_ns
    last_results = res

    out = np.empty((B, SQ, C), dtype=np.float32)
    for c in range(NCORES):
        b = c // (NCORES // B)
        qs = c % (NCORES // B)
        out[b, qs * QPC:(qs + 1) * QPC] = res.results[c]["out"]
    return out.reshape(B, 1, 32, 32, C)



# revision 10
# speedup vs baseline: 1.1419x; 1.1419x over previous
"""Fused co-memory cross-attention kernel for Trainium2, SPMD over 8 NeuronCores.

Module: LayerNorm(q/k/v) -> per-head projections -> masked softmax attention
        -> output projection.  B=2, Sq=1024, Sk=5*1024, C=256, 8 heads x 32.

Sharding: data-parallel over batch (2) x query-block parallel (4) = 8 cores.
Each core: 256 query rows of one batch vs that batch's mask-compacted key set
(masked frames dropped on host; -1e9 bias pads frames to equalize SPMD shape).

v2 design (vs the transpose-heavy v1):
  - K/V arrive HOST-TRANSPOSED (kT/vT [C, Sk] fp16) -> zero PE transposes,
    no bn_stats on K/V.
  - Per-token mean via M=1 matmuls (lhsT = -ones/C col, LDW P=1 is ~free);
    -mu rows are cast to f16, bounced through DRAM into partition-broadcast
    tiles, and kT/vT are centered in-place on VectorE.
  - Per-token var via squares of centered kT/vT + M=tok matmuls
    (lhsT = squared tile, rhs = ones/C) -> rstd columns [tok,1] via quake
    rsqrt on VectorE.
  - K's rstd is folded into the softmax through the ScalarE activation SCALE
    port (exp(rstd_t * s + frame_bias)); the bm@Wk score bias is dropped
    (softmax-invariant); V's rstd is folded into the v-projection PSUM drain.
  - vh carries a ones-column per head -> PV matmuls (M=33) accumulate the
    softmax denominator as ctx row 32 (no separate denominator matmuls).
  - Scores: K=32 matmuls, tile_position=(32h,0) row groups, consecutive
    matmuls alternate PSUM banks (measured ~2x array-level concurrency).
    PV: M=33 col strips at (0,0)/(0,64), alternating ctx banks.
  - ScalarE does ONLY exp; kp drains on GpSimd; everything else VectorE.
  - Normalize via PE broadcast of 1/den (K=1 matmul), no DRAM bounce.
  - Q path: row-major LN (bn_stats), DMA-xbar transpose (no identity matmuls).
"""

import math
import os

import numpy as np

HEADS = 8
KD = 32
C = 256
EPS = 1e-3
B = 2
SQ = 1024
FTOK = 1024
TK = 5
NCORES = 8
QPC = SQ // (NCORES // B)
NEG = -1.0e9
P = 128

_cache: dict = {}

last_exec_time_ns = None
last_results = None


def _build_program(F: int, fp16: bool = True):
    from contextlib import ExitStack

    import concourse.bass as bass
    import concourse.tile as tile
    from concourse import bacc, mybir

    dt = mybir.dt
    f32 = dt.float32
    mdt = dt.float16 if fp16 else dt.float32
    AF = mybir.ActivationFunctionType
    OP = mybir.AluOpType
    i32 = dt.int32
    SK = F * FTOK
    NT = SK // P
    NCH = NT // 4            # 512-token chunks

    nc = bacc.Bacc("TRN2", target_bir_lowering=False, debug=False,
                   num_devices=NCORES)

    xq_d = nc.dram_tensor("xq", [QPC, C], mdt, kind="ExternalInput").ap()
    kT_d = nc.dram_tensor("kT", [2, P, SK], mdt, kind="ExternalInput").ap()
    vT_d = nc.dram_tensor("vT", [2, P, SK], mdt, kind="ExternalInput").ap()
    wq_d = nc.dram_tensor("wq", [C, C], mdt, kind="ExternalInput").ap()
    wk_d = nc.dram_tensor("wk", [C, C], mdt, kind="ExternalInput").ap()
    wv_d = nc.dram_tensor("wv", [C, C], mdt, kind="ExternalInput").ap()
    wo_d = nc.dram_tensor("wo", [C, C], mdt, kind="ExternalInput").ap()
    tq_d = nc.dram_tensor("tq", [C, 1], f32, kind="ExternalInput").ap()
    tv_d = nc.dram_tensor("tv", [C, 1], f32, kind="ExternalInput").ap()
    fb_d = nc.dram_tensor("fbias", [1, F], f32, kind="ExternalInput").ap()
    out_d = nc.dram_tensor("out", [QPC, C], f32, kind="ExternalOutput").ap()
    mu_d = nc.dram_tensor("mu_scr", [2, SK], mdt).ap()
    qt_d = nc.dram_tensor("qt_scr", [QPC, C], mdt).ap()

    with tile.TileContext(nc) as tc, ExitStack() as ctx:
        singles = ctx.enter_context(tc.tile_pool(name="singles", bufs=1))
        io_p = ctx.enter_context(tc.tile_pool(name="io", bufs=4))
        sq_p = ctx.enter_context(tc.tile_pool(name="sqp", bufs=3))
        exp_p = ctx.enter_context(tc.tile_pool(name="exp", bufs=4))
        out_p = ctx.enter_context(tc.tile_pool(name="outp", bufs=2))
        stats_p = ctx.enter_context(tc.tile_pool(name="stats", bufs=4))
        ps_ctx = ctx.enter_context(
            tc.tile_pool(name="ps_ctx", bufs=1, space="PSUM"))
        ps_sc = ctx.enter_context(
            tc.tile_pool(name="ps_sc", bufs=2, space="PSUM"))

        def sct(name):
            return ps_sc.tile([P, 1024], f32, tag="sc", name=name)

        # ---- constants / weights ----
        negmc = singles.tile([P, 1], mdt)
        nc.vector.memset(negmc[:], -1.0 / C)
        posc = singles.tile([P, 1], mdt)
        nc.vector.memset(posc[:], 1.0 / C)
        ones32 = singles.tile([1, 32], mdt)
        nc.vector.memset(ones32[:], 1.0)
        fb_t = singles.tile([P, F], f32)
        nc.sync.dma_start(
            out=fb_t[:],
            in_=bass.AP(tensor=fb_d.tensor, offset=fb_d.offset,
                        ap=[[0, P], [1, F]]))
        tq_t = singles.tile([P, 2], f32, tag="tq")
        nc.sync.dma_start(
            out=tq_t[:], in_=tq_d.rearrange("(a p) o -> p (a o)", p=P))
        tv_t = singles.tile([P, 2], f32, tag="tv")
        nc.sync.dma_start(
            out=tv_t[:], in_=tv_d.rearrange("(a p) o -> p (a o)", p=P))

        w_tiles = {}
        for name, d in (("wq", wq_d), ("wk", wk_d), ("wv", wv_d), ("wo", wo_d)):
            for kt in range(2):
                t = singles.tile([P, C], mdt, tag=f"{name}{kt}")
                nc.sync.dma_start(out=t[:], in_=d[kt * P:(kt + 1) * P, :])
                w_tiles[(name, kt)] = t

        kTt = [singles.tile([P, SK], mdt, tag=f"kT{h}", name=f"kT{h}")
               for h in range(2)]
        vTt = [singles.tile([P, SK], mdt, tag=f"vT{h}", name=f"vT{h}")
               for h in range(2)]
        for h in range(2):
            nc.sync.dma_start(out=kTt[h][:], in_=kT_d[h])
            nc.sync.dma_start(out=vTt[h][:], in_=vT_d[h])

        kp = [singles.tile([64, SK], mdt, tag=f"kp{i}", name=f"kp{i}")
              for i in range(4)]
        vh = singles.tile([P, NT, HEADS, 33], mdt, tag="vh")
        nc.vector.memset(vh[:], 1.0)
        rstd_k = singles.tile([P, NT], f32, tag="rstd_k")
        rstd_v = singles.tile([P, NT], f32, tag="rstd_v")
        mub = [singles.tile([P, SK], mdt, tag=f"mub{i}", name=f"mub{i}")
               for i in range(2)]
        murow = singles.tile([1, 2, SK], mdt, tag="murow")

        def quake_rsqrt(dst_ap, var_ap, n):
            ve = stats_p.tile([P, 8], f32, tag="ve", name="ve")
            nc.vector.tensor_scalar(ve[:, 0:n], var_ap, scalar1=EPS,
                                    scalar2=None, op0=OP.add)
            y = stats_p.tile([P, 8], f32, tag="y0", name="y0")
            nc.vector.tensor_scalar(y[:, 0:n].bitcast(i32),
                                    ve[:, 0:n].bitcast(i32),
                                    scalar1=1, scalar2=None,
                                    op0=OP.logical_shift_right)
            nc.vector.tensor_scalar(y[:, 0:n].bitcast(i32),
                                    y[:, 0:n].bitcast(i32),
                                    scalar1=-1, scalar2=0x5F3759DF,
                                    op0=OP.mult, op1=OP.add)
            c1 = stats_p.tile([P, 8], f32, tag="nc", name="nwt")
            nc.vector.tensor_mul(c1[:, 0:n], y[:, 0:n], y[:, 0:n])
            nc.vector.tensor_mul(c1[:, 0:n], c1[:, 0:n], ve[:, 0:n])
            nc.vector.tensor_scalar(c1[:, 0:n], c1[:, 0:n], scalar1=-0.5,
                                    scalar2=1.5, op0=OP.mult, op1=OP.add)
            y2 = stats_p.tile([P, 8], f32, tag="yn", name="yn")
            nc.vector.tensor_mul(y2[:, 0:n], y[:, 0:n], c1[:, 0:n])
            c2 = stats_p.tile([P, 8], f32, tag="nc2", name="nwt2")
            nc.vector.tensor_mul(c2[:, 0:n], y2[:, 0:n], y2[:, 0:n])
            nc.vector.tensor_mul(c2[:, 0:n], c2[:, 0:n], ve[:, 0:n])
            nc.vector.tensor_scalar(c2[:, 0:n], c2[:, 0:n], scalar1=-0.5,
                                    scalar2=1.5, op0=OP.mult, op1=OP.add)
            nc.vector.tensor_mul(dst_ap, y2[:, 0:n], c2[:, 0:n])

        # ---- emission helpers (3-stage software pipeline) ----
        def emit_means(ch):
            cs = ch * 4 * P
            mean_ps = sct("mean_ps")
            for half in range(2):
                nc.tensor.matmul(mean_ps[0:1, 0:512],
                                 negmc[:], kTt[half][:, cs:cs + 4 * P],
                                 start=(half == 0), stop=(half == 1),
                                 skip_group_check=True)
            for half in range(2):
                nc.tensor.matmul(mean_ps[32:33, 0:512],
                                 negmc[:], vTt[half][:, cs:cs + 4 * P],
                                 start=(half == 0), stop=(half == 1),
                                 skip_group_check=True)
            nc.vector.tensor_copy(murow[:, 0, cs:cs + 4 * P],
                                  mean_ps[0:1, 0:512])
            nc.vector.tensor_copy(murow[:, 1, cs:cs + 4 * P],
                                  mean_ps[32:33, 0:512])
            nc.sync.dma_start(out=mu_d[0:1, cs:cs + 4 * P],
                              in_=murow[:, 0, cs:cs + 4 * P])
            nc.sync.dma_start(out=mu_d[1:2, cs:cs + 4 * P],
                              in_=murow[:, 1, cs:cs + 4 * P])
            for i in range(2):
                nc.gpsimd.dma_start(
                    out=mub[i][:, cs:cs + 4 * P],
                    in_=mu_d[i:i + 1, cs:cs + 4 * P].partition_broadcast(P))

        def emit_center(ch):
            cs = ch * 4 * P
            for half in range(2):
                nc.vector.tensor_tensor(
                    kTt[half][:, cs:cs + 4 * P], kTt[half][:, cs:cs + 4 * P],
                    mub[0][:, cs:cs + 4 * P], op=OP.add)
                nc.vector.tensor_tensor(
                    vTt[half][:, cs:cs + 4 * P], vTt[half][:, cs:cs + 4 * P],
                    mub[1][:, cs:cs + 4 * P], op=OP.add)

        def emit_kp(ch, mt):
            cs = ch * 4 * P
            kps = sct("kp_ps")
            for kt in range(2):
                nc.tensor.matmul(
                    kps[:, 0:512],
                    w_tiles[("wk", kt)][:, mt * P:(mt + 1) * P],
                    kTt[kt][:, cs:cs + 4 * P],
                    start=(kt == 0), stop=(kt == 1),
                    skip_group_check=True)
            nc.vector.tensor_copy(kp[2 * mt][:, cs:cs + 4 * P],
                                  kps[0:64, 0:512])
            nc.vector.tensor_copy(kp[2 * mt + 1][:, cs:cs + 4 * P],
                                  kps[64:128, 0:512])

        def emit_sq(ch):
            cs = ch * 4 * P
            sqk = sq_p.tile([P, 512], mdt, tag="sqk", name="sqk")
            sqv = sq_p.tile([P, 512], mdt, tag="sqv", name="sqv")
            sqt = sq_p.tile([P, 512], mdt, tag="sqt", name="sqt")
            nc.vector.tensor_mul(sqk[:], kTt[0][:, cs:cs + 4 * P],
                                 kTt[0][:, cs:cs + 4 * P])
            nc.vector.tensor_mul(sqt[:], kTt[1][:, cs:cs + 4 * P],
                                 kTt[1][:, cs:cs + 4 * P])
            nc.vector.tensor_tensor(sqk[:], sqk[:], sqt[:], op=OP.add)
            nc.vector.tensor_mul(sqv[:], vTt[0][:, cs:cs + 4 * P],
                                 vTt[0][:, cs:cs + 4 * P])
            nc.vector.tensor_mul(sqt[:], vTt[1][:, cs:cs + 4 * P],
                                 vTt[1][:, cs:cs + 4 * P])
            nc.vector.tensor_tensor(sqv[:], sqv[:], sqt[:], op=OP.add)
            sqs = sct("sq_ps")
            for j in range(4):
                nc.tensor.matmul(sqs[:, j:j + 1],
                                 sqk[:, j * P:(j + 1) * P], posc[:],
                                 start=True, stop=True, skip_group_check=True)
                nc.tensor.matmul(sqs[:, 4 + j:5 + j],
                                 sqv[:, j * P:(j + 1) * P], posc[:],
                                 start=True, stop=True, skip_group_check=True)
            var8 = stats_p.tile([P, 8], f32, tag="var8", name="var8")
            nc.vector.tensor_copy(var8[:], sqs[:, 0:8])
            quake_rsqrt(rstd_k[:, 4 * ch:4 * ch + 4], var8[:, 0:4], 4)
            quake_rsqrt(rstd_v[:, 4 * ch:4 * ch + 4], var8[:, 4:8], 4)

        def emit_v(ch, j):
            t = 4 * ch + j
            ts = t * P
            vps = sct("v_ps")
            for kt in range(2):
                nc.tensor.matmul(
                    vps[:, 0:256],
                    vTt[kt][:, ts:ts + P],
                    w_tiles[("wv", kt)][:],
                    start=(kt == 0), stop=(kt == 1),
                    skip_group_check=True)
            nc.vector.tensor_scalar(
                vh[:, t, :, 0:32],
                bass.AP(tensor=vps.tensor, offset=vps.offset,
                        ap=[[1024, P], [32, 8], [1, 32]]),
                scalar1=rstd_v[:, t:t + 1], scalar2=None, op0=OP.mult)

        # ---- Q path (emitted first; q-proj matmuls deferred to round -2) ----
        xts = []
        for qt in range(2):
            xt = io_p.tile([P, C], mdt, tag="xq", name="xt")
            nc.sync.dma_start(out=xt[:], in_=xq_d[qt * P:(qt + 1) * P, :])
            xts.append(xt)
        mvb = stats_p.tile([P, 2, 2], f32, tag="mv", name="mvb")
        for i, x_t in enumerate(xts):
            st = stats_p.tile([P, 6], f32, tag="bn", name="st")
            nc.vector.bn_stats(st[:], x_t[:])
            nc.vector.bn_aggr(mvb[:, i, :], st[:])
        rstdq = stats_p.tile([P, 2], f32, tag="rstdq", name="rstdq")
        quake_rsqrt(rstdq[:, 0:2], mvb[:, :, 1], 2)
        xqT = singles.tile([P, 2, 2 * P], mdt, tag="xqT")
        for qt in range(2):
            nmr = stats_p.tile([P, 1], f32, tag="nmr", name="nmr")
            nc.vector.tensor_scalar(nmr[:], mvb[:, qt, 0:1],
                                    scalar1=rstdq[:, qt:qt + 1],
                                    scalar2=-1.0, op0=OP.mult, op1=OP.mult)
            xh = io_p.tile([P, C], mdt, tag="xh", name="xh")
            nc.vector.tensor_scalar(xh[:], xts[qt][:],
                                    scalar1=rstdq[:, qt:qt + 1],
                                    scalar2=nmr[:], op0=OP.mult, op1=OP.add)
            nc.sync.dma_start(out=qt_d[qt * P:(qt + 1) * P, :], in_=xh[:])
        for ct in range(2):
            nc.sync.dma_start_transpose(
                out=xqT[:, ct, :], in_=qt_d[:, ct * P:(ct + 1) * P])

        qp = [singles.tile([64, 2 * P], mdt, tag=f"qp{i}", name=f"qp{i}")
              for i in range(4)]

        def emit_qproj(mt):
            qps = sct("q_ps")
            for kt in range(2):
                nc.tensor.matmul(
                    qps[:, 0:256],
                    w_tiles[("wq", kt)][:, mt * P:(mt + 1) * P],
                    xqT[:, kt, :], start=(kt == 0), stop=(kt == 1),
                    skip_group_check=True)
            for half in range(2):
                nc.vector.tensor_scalar(
                    qp[2 * mt + half][:], qps[64 * half:64 * half + 64, 0:256],
                    scalar1=tq_t[64 * half:64 * half + 64, mt:mt + 1],
                    scalar2=None, op0=OP.add)

        # ---- attention slot emitters ----
        ctx_ps = [ps_ctx.tile([P, 512], f32, tag=f"ctx{i}", name=f"ctx{i}")
                  for i in range(4)]

        def emit_scores(t, quad):
            ts = t * P
            f = t // (FTOK // P)
            sc = sct("sc")
            for p, e in ((0, 0), (0, 1), (1, 0), (1, 1)):
                slot = 2 * e + p
                nc.tensor.matmul(
                    sc[:, slot * 256:(slot + 1) * 256],
                    kp[2 * quad + p][32 * e:32 * e + 32, ts:ts + P],
                    qp[2 * quad + p][32 * e:32 * e + 32, :],
                    start=True, stop=True, tile_position=(32 * e, 0),
                    skip_group_check=True)
            ex = exp_p.tile([P, 1024], mdt, tag="exp")
            nc.scalar.activation(ex[:], sc[:], AF.Exp,
                                 bias=fb_t[:, f:f + 1],
                                 scale=rstd_k[:, t:t + 1])
            return ex

        def emit_pv(t, quad, ex):
            for j in range(4):
                h = 4 * quad + j
                slot = 2 * (j % 2) + j // 2
                nc.tensor.matmul(
                    ctx_ps[2 * quad + j // 2][
                        64 * (j % 2):64 * (j % 2) + 33, 0:256],
                    vh[:, t, h, 0:33],
                    ex[:, slot * 256:(slot + 1) * 256],
                    start=(t == 0), stop=(t == NT - 1),
                    tile_position=(0, 64 * (j % 2)),
                    skip_group_check=True)

        # ---- pipelined emission ----
        pending = None
        for r in range(-2, NCH):
            pieces = []
            if r + 2 < NCH:
                pieces.append(lambda c=r + 2: emit_means(c))
            if r == -2:
                pieces.append(lambda: emit_qproj(0))
                pieces.append(lambda: emit_qproj(1))
            if 0 <= r + 1 < NCH:
                pieces.append(lambda c=r + 1: emit_center(c))
                pieces.append(lambda c=r + 1: emit_kp(c, 0))
                pieces.append(lambda c=r + 1: emit_kp(c, 1))
                pieces.append(lambda c=r + 1: emit_sq(c))
                for j in range(4):
                    pieces.append(lambda c=r + 1, jj=j: emit_v(c, jj))
            slots = ([(t, quad) for t in range(4 * r, 4 * r + 4)
                      for quad in range(2)] if r >= 0 else [])
            n = max(len(pieces), len(slots))
            for i in range(n):
                if i < len(pieces):
                    pieces[i]()
                if i < len(slots):
                    t, quad = slots[i]
                    ex = emit_scores(t, quad)
                    if pending is not None:
                        emit_pv(*pending)
                    pending = (t, quad, ex)
        if pending is not None:
            emit_pv(*pending)

        # ---- normalize: ctx/den via PE broadcast of reciprocal ----
        ctxn = [singles.tile([P, 2 * P], mdt, tag=f"ctxn{q}", name=f"ctxn{q}")
                for q in range(2)]
        for q in range(2):
            for j in range(4):
                bank = ctx_ps[2 * q + j // 2]
                base = 64 * (j % 2)
                rden = out_p.tile([1, 256], mdt, tag="rden", name="rden")
                with nc.allow_low_precision(reason="1/den in f16 matches f16 data path"):
                    nc.vector.reciprocal(rden[:], bank[base + 32:base + 33, 0:256])
                rps = sct("rden_ps")
                nc.tensor.matmul(rps[0:32, 0:256], ones32[:], rden[:],
                                 start=True, stop=True, skip_group_check=True)
                rbc = out_p.tile([32, 256], mdt, tag="rbc", name="rbc")
                nc.vector.tensor_copy(rbc[:], rps[0:32, 0:256])
                nc.vector.tensor_tensor(
                    ctxn[q][32 * j:32 * j + 32, :],
                    bank[base:base + 32, 0:256], rbc[:], op=OP.mult)
            nc.vector.tensor_scalar(ctxn[q][:], ctxn[q][:],
                                    scalar1=tv_t[:, q:q + 1],
                                    scalar2=None, op0=OP.add)

        # ---- output projection ----
        for qt in range(2):
            ops = sct("o_ps")
            for kt in range(2):
                nc.tensor.matmul(
                    ops[:, 0:256],
                    ctxn[kt][:, qt * P:(qt + 1) * P],
                    w_tiles[("wo", kt)][:],
                    start=(kt == 0), stop=(kt == 1), skip_group_check=True)
            ot = out_p.tile([P, C], f32, tag="ot")
            nc.vector.tensor_copy(ot[:], ops[:, 0:256])
            nc.sync.dma_start(out=out_d[qt * P:(qt + 1) * P, :], in_=ot[:])

    nc.compile()
    return nc


def _get_program(F: int, use_tk: bool = False, fp16: bool = True):
    key = (F, fp16)
    if key not in _cache:
        _cache[key] = _build_program(F, fp16)
    return _cache[key]


def _prep_host(encoder_output, memory_key, memory_value, Wq, Wk, Wv, Wo,
               gamma_q, beta_q, gamma_m, beta_m, memory_mask, fp16=True):
    f32 = np.float32
    mdt = np.float16 if fp16 else np.float32
    enc = np.ascontiguousarray(
        np.asarray(encoder_output, dtype=f32).reshape(B, SQ, C))
    mk = np.asarray(memory_key, dtype=f32).reshape(B, TK, FTOK, C)
    mv = np.asarray(memory_value, dtype=f32).reshape(B, TK, FTOK, C)
    mask = np.asarray(memory_mask).astype(np.int64)

    gq = np.asarray(gamma_q, dtype=f32)
    bq = np.asarray(beta_q, dtype=f32)
    gm = np.asarray(gamma_m, dtype=f32)
    bm = np.asarray(beta_m, dtype=f32)
    Wq = np.asarray(Wq, dtype=f32)
    Wk = np.asarray(Wk, dtype=f32)
    Wv = np.asarray(Wv, dtype=f32)
    Wo = np.ascontiguousarray(np.asarray(Wo, dtype=f32))

    s = 1.0 / math.sqrt(KD)
    wq2 = np.ascontiguousarray(gq[:, None] * Wq * s)
    tq = np.ascontiguousarray((bq @ Wq * s).reshape(C, 1))
    wk2 = np.ascontiguousarray(gm[:, None] * Wk)
    wv2 = np.ascontiguousarray(gm[:, None] * Wv)
    tv = np.ascontiguousarray((bm @ Wv).reshape(C, 1))
    # bm @ Wk score bias is softmax-invariant -> dropped.

    sel = []
    counts = []
    for b in range(B):
        act = np.nonzero(mask[b])[0]
        if len(act) == 0:
            sel.append((list(range(TK)), True))
            counts.append(TK)
        else:
            sel.append((list(act), False))
            counts.append(len(act))
    F = max(counts)

    per_batch = []
    for b in range(B):
        frames, uniform = sel[b]
        fb = np.zeros((1, F), dtype=f32)
        fr = list(frames)
        while len(fr) < F:
            fr.append(frames[-1])
            fb[0, len(fr) - 1] = NEG
        kb = mk[b][fr].reshape(F * FTOK, C)
        vb = mv[b][fr].reshape(F * FTOK, C)
        kbT = np.ascontiguousarray(kb.T.reshape(2, P, F * FTOK))
        vbT = np.ascontiguousarray(vb.T.reshape(2, P, F * FTOK))
        if uniform:
            wq_b = np.zeros_like(wq2)
            tq_b = np.zeros_like(tq)
        else:
            wq_b = wq2
            tq_b = tq
        per_batch.append(dict(kT=kbT.astype(mdt), vT=vbT.astype(mdt),
                              wq=wq_b.astype(mdt), tq=tq_b, fbias=fb))

    in_maps = []
    for c in range(NCORES):
        b = c // (NCORES // B)
        qs = c % (NCORES // B)
        m = dict(per_batch[b])
        m["xq"] = np.ascontiguousarray(enc[b, qs * QPC:(qs + 1) * QPC]).astype(mdt)
        m["wk"] = wk2.astype(mdt)
        m["wv"] = wv2.astype(mdt)
        m["wo"] = Wo.astype(mdt)
        m["tv"] = tv
        in_maps.append(m)
    return F, False, in_maps


def kernel(encoder_output, memory_key, memory_value, Wq, Wk, Wv, Wo,
           gamma_q, beta_q, gamma_m, beta_m, memory_mask):
    global last_exec_time_ns, last_results
    from concourse.bass_utils import run_bass_kernel_spmd

    fp16 = os.environ.get("KERNEL_FP32", "0") != "1"
    F, use_tk, in_maps = _prep_host(
        encoder_output, memory_key, memory_value, Wq, Wk, Wv, Wo,
        gamma_q, beta_q, gamma_m, beta_m, memory_mask, fp16=fp16)
    nc = _get_program(F, use_tk, fp16)

    trace = os.environ.get("BASS_KERNEL_TRACE", "0") == "1"
    res = run_bass_kernel_spmd(nc, in_maps, core_ids=list(range(NCORES)),
                               trace=trace)
    last_exec_time_ns = res.exec_time_ns
    last_results = res

    out = np.empty((B, SQ, C), dtype=np.float32)
    for c in range(NCORES):
        b = c // (NCORES // B)
        qs = c % (NCORES // B)
        out[b, qs * QPC:(qs + 1) * QPC] = res.results[c]["out"]
    return out.reshape(B, 1, 32, 32, C)
